# revision 9
# baseline (speedup 1.0000x reference)
"""MoE routing kernel v2.1 for Trainium2 — grouped GEMM, 8 NeuronCores.

Sharding: 2-way expert-parallel x 4-way data-parallel. Core c handles data
group g = c//2 (2048 tokens) and expert half h = c%2 (5 of the 10 experts).
The gate matrix columns (and the expert weights) are host-permuted per core
so local experts are always gating columns 0..4 — one NEFF serves all cores.

Per core, on device:
  1. PE-transpose x -> xT, gating matmuls, softmax, top-2 (max8), one-hot
     top-1/top-2 masks via value equality (all exact fp32), per-tile ranks
     via triangular matmuls (interleaved with gating).
  2. Cross-tile rank offsets: one totals matmul + Hillis-Steele prefix on
     partition 0 + partition_broadcast.
  3. Tokens scatter [token, 2*token+slot] rows into a DRAM tokmap at their
     capacity-row index via indirect DMA (2 batched calls); reading it back
     row-ordered yields gather/scatter indices. Padding rows were prefilled
     with [0, BIG]: they gather token 0 (finite) and scatter out-of-bounds
     (silently skipped).
  4. Per local expert: batched indirect-DMA row gather, PE-transpose,
     grouped GEMM (fp32r, 1 cycle/col) + bias, batched indirect-DMA row
     scatter into the [2*T, H] output. Pair cores write disjoint rows;
     host sums the pair outputs.

Capacity per local expert (C) is sized at build time from a cheap host-side
gating estimate of the actual input (device still computes all routing).
"""

import os

import numpy as np

import concourse.bass as bass
import concourse.mybir as mybir
import concourse.tile as tile
from concourse import bacc
from concourse.bass import ts
from concourse.bass_utils import run_bass_kernel_spmd

f32 = mybir.dt.float32
f32r = mybir.dt.float32r
i32 = mybir.dt.int32

B, S, E, H, NE = 4, 2048, 1024, 2048, 10
NCORES = 8
NGROUP = 4              # data groups
NLOC = NE // 2          # local experts per core = 5
T2 = (B * S) // NGROUP  # tokens per core = 2048
TT = T2 // 128          # token tiles = 16
EJ = E // 128           # contraction subtiles = 8
CGW = 512
CG = H // CGW           # column groups = 4
BIG = 1.0e7

LAST_RESULTS = None


def _build_program(C):
    RC = C // 128  # row chunks per local expert
    nc = bacc.Bacc(None, target_bir_lowering=False, debug=False)

    xs = nc.dram_tensor("xs", [T2, E], f32, kind="ExternalInput")
    gwT = nc.dram_tensor("gwT", [128, EJ, NE], f32, kind="ExternalInput")
    gbB = nc.dram_tensor("gbB", [128, NE], f32, kind="ExternalInput")
    ew5 = nc.dram_tensor("ew5", [NLOC, E, H], f32, kind="ExternalInput")
    ebB5 = nc.dram_tensor("ebB5", [128, NLOC, H], f32, kind="ExternalInput")
    ident = nc.dram_tensor("ident", [128, 128], f32, kind="ExternalInput")
    tri = nc.dram_tensor("tri", [128, 128], f32, kind="ExternalInput")
    ones2d = nc.dram_tensor("ones2d", [128, 128], f32, kind="ExternalInput")
    toki = nc.dram_tensor("toki", [128, TT], f32, kind="ExternalInput")
    lCm1 = nc.dram_tensor("lCm1", [128, NLOC], f32, kind="ExternalInput")
    ow = nc.dram_tensor("ow", [T2, 2], f32, kind="ExternalOutput")
    oe = nc.dram_tensor("oe", [2 * T2, H], f32, kind="ExternalOutput")

    with tile.TileContext(nc) as tc:
        with (
            tc.tile_pool(name="const", bufs=1) as const,
            tc.tile_pool(name="masks", bufs=1) as mp,
            tc.tile_pool(name="route", bufs=1) as rp,
            tc.tile_pool(name="dram", bufs=1, space="DRAM") as dramp,
            tc.tile_pool(name="wp", bufs=3) as wp,
            tc.tile_pool(name="bp", bufs=2) as bp,
        ):
            ident_sb = const.tile([128, 128], f32)
            nc.sync.dma_start(ident_sb[:], ident[:])
            tri_sb = const.tile([128, 128], f32)
            nc.sync.dma_start(tri_sb[:], tri[:])
            ones_sb = const.tile([128, 128], f32)
            nc.sync.dma_start(ones_sb[:], ones2d[:])
            toki_sb = const.tile([128, TT], f32)
            nc.sync.dma_start(toki_sb[:], toki[:])
            lCm1_sb = const.tile([128, NLOC], f32)
            nc.sync.dma_start(lCm1_sb[:], lCm1[:])
            gw_sb = const.tile([128, EJ, NE], f32)
            nc.sync.dma_start(gw_sb[:], gwT[:])
            gb_sb = const.tile([128, NE], f32)
            nc.sync.dma_start(gb_sb[:], gbB[:])

            tokmap = dramp.tile([NLOC * C, 2], f32)
            # prefill: token=0 (valid), dst=BIG (scatter-skipped)
            fill = rp.tile([128, NLOC * C // 128, 2], f32, tag="fill")
            nc.vector.memset(fill[:, :, 0:1], 0.0)
            nc.vector.memset(fill[:, :, 1:2], BIG)
            nc.sync.dma_start(
                tokmap[:].rearrange("(c p) v -> p c v", p=128), fill[:]
            )
            # vv[k][:, tt, :] = [token, 2*token + k]
            vvk = []
            for k in range(2):
                vv = rp.tile([128, TT, 2], f32, tag=f"vv{k}")
                nc.vector.tensor_copy(vv[:, :, 0:1], toki_sb[:, :, None])
                nc.vector.tensor_scalar(
                    vv[:, :, 1:2], toki_sb[:, :, None], 2.0, float(k),
                    op0=mybir.AluOpType.mult, op1=mybir.AluOpType.add,
                )
                vvk.append(vv)
            offs_cur = rp.tile([1, NE], f32, tag="offs_cur")
            nc.vector.memset(offs_cur[:], 0.0)

            m0f = mp.tile([128, TT, NE], f32)
            m1f = mp.tile([128, TT, NE], f32)
            m01 = mp.tile([128, TT, NE], f32)
            wstage = mp.tile([128, TT, 2], f32)
            tok32 = rp.tile([128, NLOC, RC], i32)  # gather row -> source token
            dst32 = rp.tile([128, NLOC, RC], i32)  # gather row -> output row

            # ===== phase A: gating / top-2 / per-tile ranks =====
            with (
                tc.tile_pool(name="xT", bufs=1) as xtp,
                tc.tile_pool(name="xin", bufs=2) as xpool,
                tc.tile_pool(name="tpsA", bufs=2, space="PSUM") as tpsA,
                tc.tile_pool(name="gps", bufs=2, space="PSUM") as gps_p,
                tc.tile_pool(name="cps", bufs=1, space="PSUM") as cps_p,
                tc.tile_pool(name="offs", bufs=1, space="PSUM") as offs_p,
                tc.tile_pool(name="smx", bufs=2) as spool,
            ):
                xT_sb = xtp.tile([128, EJ, T2], f32)
                offs_ps = offs_p.tile([128, NE], f32)
                # ~5us of dummy matmuls to warm the PE clock gate (HAM)
                wps = tpsA.tile([128, 4, 128], f32, tag="ps")
                for _ in range(12):
                    nc.tensor.matmul(
                        wps[:, 0, :], lhsT=ones_sb[:], rhs=ones_sb[:],
                        start=True, stop=True, skip_group_check=True,
                    )
                for tt in range(TT):
                    xt = xpool.tile([128, E], f32)
                    nc.sync.dma_start(xt[:], xs[ts(tt, 128), :])
                    for jh in range(2):
                        ps = tpsA.tile([128, 4, 128], f32)
                        for jj in range(4):
                            nc.tensor.transpose(
                                ps[:, jj, :], xt[:, ts(jh * 4 + jj, 128)],
                                ident_sb[:],
                            )
                        nc.vector.tensor_copy(
                            xT_sb[:, jh * 4 : jh * 4 + 4, ts(tt, 128)], ps[:]
                        )
                    gps = gps_p.tile([128, NE], f32)
                    # bias via K=1 matmul, then accumulate the gating products
                    nc.tensor.matmul(
                        gps[:], lhsT=ones_sb[0:1, :], rhs=gb_sb[0:1, :],
                        start=True, stop=False,
                    )
                    for j in range(EJ):
                        nc.tensor.matmul(
                            gps[:],
                            lhsT=xT_sb[:, j, ts(tt, 128)],
                            rhs=gw_sb[:, j, :],
                            start=False,
                            stop=(j == EJ - 1),
                        )
                    scores = spool.tile([128, NE], f32)
                    nc.vector.tensor_copy(scores[:], gps[:])
                    v8 = spool.tile([128, 8], f32, tag="v8")
                    nc.vector.max(v8[:], scores[:])
                    nc.vector.tensor_tensor(
                        m0f[:, tt, :], scores[:], v8[:, 0:1].to_broadcast([128, NE]),
                        op=mybir.AluOpType.is_equal,
                    )
                    nc.vector.tensor_tensor(
                        m1f[:, tt, :], scores[:], v8[:, 1:2].to_broadcast([128, NE]),
                        op=mybir.AluOpType.is_equal,
                    )
                    nc.vector.tensor_add(m01[:, tt, :], m0f[:, tt, :], m1f[:, tt, :])
                    # top-2 softmax weights in closed form:
                    # w1 = 1/sum(exp(s - v1)), w2 = exp(v2 - v1) * w1
                    nv1 = spool.tile([128, 1], f32, tag="nv1")
                    nc.vector.tensor_scalar_mul(nv1[:], v8[:, 0:1], -1.0)
                    exps = spool.tile([128, NE], f32, tag="exps")
                    ssum = spool.tile([128, 1], f32, tag="ssum")
                    nc.scalar.activation(
                        exps[:],
                        scores[:],
                        mybir.ActivationFunctionType.Exp,
                        bias=nv1[:],
                        scale=1.0,
                        accum_out=ssum[:],
                    )
                    nc.vector.reciprocal(wstage[:, tt, 0:1], ssum[:])
                    d21 = spool.tile([128, 1], f32, tag="d21")
                    nc.vector.tensor_add(d21[:], v8[:, 1:2], nv1[:])
                    e21 = spool.tile([128, 1], f32, tag="e21")
                    nc.scalar.activation(
                        e21[:], d21[:], mybir.ActivationFunctionType.Exp
                    )
                    nc.vector.tensor_tensor(
                        wstage[:, tt, 1:2], e21[:], wstage[:, tt, 0:1],
                        op=mybir.AluOpType.mult,
                    )
                    # within-tile inclusive rank + cross-tile offset (psum)
                    cps = cps_p.tile([128, NE], f32)
                    nc.tensor.matmul(
                        cps[:], lhsT=tri_sb[:], rhs=m01[:, tt, :],
                        start=True, stop=True,
                    )
                    posi_t = spool.tile([128, NE], f32, tag="posi_t")
                    if tt == 0:
                        nc.vector.tensor_copy(posi_t[:], cps[:])
                    else:
                        nc.vector.tensor_add(posi_t[:], cps[:], offs_sb[:])
                    nc.tensor.matmul(
                        offs_ps[:], lhsT=ones_sb[:], rhs=m01[:, tt, :],
                        start=(tt == 0), stop=(tt == TT - 1),
                        skip_group_check=True,
                    )
                    if tt < TT - 1:
                        offs_sb = spool.tile([128, NE], f32, tag="offs_sb")
                        nc.vector.tensor_copy(offs_sb[:], offs_ps[:])
                    # capacity-row index per slot; scatter [tok, dst] to tokmap
                    t1 = spool.tile([128, NLOC], f32, tag="t1")
                    nc.vector.tensor_add(
                        t1[:], posi_t[:, 0:NLOC], lCm1_sb[:]
                    )
                    for k in range(2):
                        mk5 = (m0f if k == 0 else m1f)[:, tt, 0:NLOC]
                        rk = spool.tile([128, 1], f32, tag=f"rk{k}")
                        t2 = spool.tile([128, NLOC], f32, tag=f"t2{k}")
                        nc.vector.tensor_tensor(
                            t2[:], t1[:], mk5, op=mybir.AluOpType.mult
                        )
                        nc.vector.tensor_reduce(
                            rk[:], t2[:], mybir.AxisListType.X,
                            mybir.AluOpType.add,
                        )
                        selk = spool.tile([128, 1], f32, tag=f"selk{k}")
                        nc.vector.tensor_reduce(
                            selk[:], mk5, mybir.AxisListType.X,
                            mybir.AluOpType.add,
                        )
                        gsel = spool.tile([128, 1], f32, tag=f"gsel{k}")
                        nc.vector.tensor_scalar(
                            gsel[:], selk[:], -BIG, BIG,
                            op0=mybir.AluOpType.mult, op1=mybir.AluOpType.add,
                        )
                        nc.vector.tensor_add(rk[:], rk[:], gsel[:])
                        rk32 = spool.tile([128, 1], i32, tag=f"rk32{k}")
                        nc.vector.tensor_copy(rk32[:], rk[:])
                        nc.gpsimd.indirect_dma_start(
                            out=tokmap[:],
                            out_offset=bass.IndirectOffsetOnAxis(
                                ap=rk32[:], axis=0
                            ),
                            in_=vvk[k][:, tt, :],
                            in_offset=None,
                            bounds_check=NLOC * C - 1,
                            oob_is_err=False,
                        )
                nc.sync.dma_start(ow.rearrange("(i p) k -> p i k", p=128), wstage[:])
                # re-warm the PE clock gate during the routing-index bubble
                wps2 = tpsA.tile([128, 4, 128], f32, tag="ps")
                for _ in range(16):
                    nc.tensor.matmul(
                        wps2[:, 0, :], lhsT=ones_sb[:], rhs=ones_sb[:],
                        start=True, stop=True, skip_group_check=True,
                    )

            # ===== phase B: read back tokmap -> (token, dst) int32 =====
            with tc.tile_pool(name="rt", bufs=1) as rtp:
                idxrow = rtp.tile([128, NLOC * RC, 2], f32, tag="idxrow")
                nc.sync.dma_start(
                    idxrow[:], tokmap[:].rearrange("(c p) v -> p c v", p=128)
                )
                nc.vector.tensor_copy(
                    tok32[:].rearrange("p a b -> p (a b)")[:, :, None],
                    idxrow[:, :, 0:1],
                )
                nc.vector.tensor_copy(
                    dst32[:].rearrange("p a b -> p (a b)")[:, :, None],
                    idxrow[:, :, 1:2],
                )

            # ===== phase E: gather -> grouped GEMM -> scatter =====
            with (
                tc.tile_pool(name="xg", bufs=2) as xgp,
                tc.tile_pool(name="xgT", bufs=2) as xgtp,
                tc.tile_pool(name="og", bufs=2) as ogp,
                tc.tile_pool(name="tpsE", bufs=2, space="PSUM") as tpsE,
                tc.tile_pool(name="mps", bufs=4, space="PSUM") as mps,
            ):
                def gather_l(l):
                    xg = xgp.tile([128, RC, E], f32, tag="xg")
                    for rc in range(RC):
                        nc.gpsimd.indirect_dma_start(
                            out=xg[:, rc, :],
                            out_offset=None,
                            in_=xs[:],
                            in_offset=bass.IndirectOffsetOnAxis(
                                ap=tok32[:, l, ts(rc, 1)], axis=0
                            ),
                        )
                    return xg

                def transpose_chunk(xg, xgT, rc):
                    for jh in range(2):
                        ps = tpsE.tile([128, 4, 128], f32)
                        for jj in range(4):
                            nc.tensor.transpose(
                                ps[:, jj, :], xg[:, rc, ts(jh * 4 + jj, 128)],
                                ident_sb[:],
                            )
                        nc.vector.tensor_copy(
                            xgT[:, jh * 4 : jh * 4 + 4, ts(rc, 128)], ps[:]
                        )

                xg_cur = gather_l(0)
                xgT_cur = xgtp.tile([128, EJ, C], f32r, tag="xgT")
                for rc in range(RC):
                    transpose_chunk(xg_cur, xgT_cur, rc)
                for l in range(NLOC):
                    xgT = xgT_cur
                    if l + 1 < NLOC:
                        xg_next = gather_l(l + 1)
                        xgT_next = xgtp.tile([128, EJ, C], f32r, tag="xgT")
                    og = ogp.tile([128, RC, H], f32)
                    for cg in range(CG):
                        wT = wp.tile([128, EJ, CGW], f32r)
                        nc.sync.dma_start(
                            wT[:],
                            ew5[l]
                            .rearrange("(j p) h -> p j h", p=128)[:, :, ts(cg, CGW)]
                            .bitcast(f32r),
                        )
                        bb = bp.tile([128, CGW], f32)
                        nc.sync.dma_start(bb[:], ebB5[:, l, ts(cg, CGW)])
                        for rc in range(RC):
                            ps = mps.tile([128, CGW], f32)
                            for j in range(EJ):
                                nc.tensor.matmul(
                                    ps[:],
                                    lhsT=xgT[:, j, ts(rc, 128)],
                                    rhs=wT[:, j, :],
                                    start=(j == 0),
                                    stop=(j == EJ - 1),
                                )
                            nc.vector.tensor_add(ps[:], ps[:], bb[:])
                            nc.scalar.copy(og[:, rc, ts(cg, CGW)], ps[:])
                        # next expert's transposes ride between matmul bursts
                        if l + 1 < NLOC and cg < RC:
                            transpose_chunk(xg_next, xgT_next, cg)
                    for rc in range(RC):
                        nc.gpsimd.indirect_dma_start(
                            out=oe[:],
                            out_offset=bass.IndirectOffsetOnAxis(
                                ap=dst32[:, l, ts(rc, 1)], axis=0
                            ),
                            in_=og[:, rc, :],
                            in_offset=None,
                            bounds_check=2 * T2 - 1,
                            oob_is_err=False,
                        )
                    if l + 1 < NLOC:
                        xg_cur = xg_next
                        xgT_cur = xgT_next

    nc.compile()
    return nc


_CACHED = {}


def _capacity(x, gate_w, gate_b):
    """Host-side gating estimate, only to size the static capacity."""
    xf = x.reshape(B * S, E)
    scores = xf @ gate_w + gate_b
    part = np.argpartition(-scores, 2, axis=1)[:, :2]
    maxc = 0
    for g in range(NGROUP):
        sl = part[g * T2 : (g + 1) * T2]
        cnt = np.bincount(sl.ravel(), minlength=NE)
        maxc = max(maxc, int(cnt.max()))
    return ((maxc + 45 + 127) // 128) * 128


def kernel(x, gate_w, gate_b, expert_w, expert_b):
    global LAST_RESULTS
    x = np.ascontiguousarray(np.asarray(x, dtype=np.float32))
    gate_w = np.ascontiguousarray(np.asarray(gate_w, dtype=np.float32))
    gate_b = np.ascontiguousarray(np.asarray(gate_b, dtype=np.float32))
    expert_w = np.ascontiguousarray(np.asarray(expert_w, dtype=np.float32))
    expert_b = np.ascontiguousarray(np.asarray(expert_b, dtype=np.float32))

    C = _capacity(x, gate_w, gate_b)
    if C not in _CACHED:
        _CACHED[C] = _build_program(C)
    nc = _CACHED[C]

    xf = x.reshape(B * S, E)
    identity = np.eye(128, dtype=np.float32)
    tri = np.triu(np.ones((128, 128), dtype=np.float32))  # tri[t', t]=1 if t'<=t
    ones2d = np.ones((128, 128), dtype=np.float32)
    lCm1 = np.broadcast_to(
        (np.arange(NLOC, dtype=np.float32) * C - 1.0)[None, :], (128, NLOC)
    ).copy()
    toki = np.empty((128, TT), dtype=np.float32)
    for tt in range(TT):
        toki[:, tt] = tt * 128 + np.arange(128)

    in_maps = []
    for c in range(NCORES):
        g, h = divmod(c, 2)
        perm = list(range(NE)) if h == 0 else list(range(NLOC, NE)) + list(
            range(NLOC)
        )
        gw_p = gate_w[:, perm]
        gwT = np.ascontiguousarray(gw_p.reshape(EJ, 128, NE).transpose(1, 0, 2))
        gbB = np.ascontiguousarray(
            np.broadcast_to(gate_b[perm][None, :], (128, NE))
        )
        ew5 = np.ascontiguousarray(expert_w[perm[:NLOC]])
        ebB5 = np.ascontiguousarray(
            np.broadcast_to(expert_b[perm[:NLOC]][None, :, :], (128, NLOC, H))
        )
        in_maps.append(
            {
                "xs": np.ascontiguousarray(xf[g * T2 : (g + 1) * T2]),
                "gwT": gwT,
                "gbB": gbB,
                "ew5": ew5,
                "ebB5": ebB5,
                "ident": identity,
                "tri": tri,
                "ones2d": ones2d,
                "toki": toki,
                "lCm1": lCm1,
            }
        )

    trace = bool(int(os.environ.get("KERNEL_TRACE", "0")))
    res = run_bass_kernel_spmd(
        nc,
        in_maps,
        core_ids=list(range(NCORES)),
        trace=trace,
        trace_cores=list(range(NCORES)) if trace else None,
    )
    LAST_RESULTS = res

    ow_full = np.concatenate(
        [res.results[2 * g]["ow"] for g in range(NGROUP)], axis=0
    )
    oe_full = np.concatenate(
        [
            (res.results[2 * g]["oe"] + res.results[2 * g + 1]["oe"]).reshape(
                T2, 2, H
            )
            for g in range(NGROUP)
        ],
        axis=0,
    )
    return (
        ow_full.reshape(B, S, 2),
        oe_full.reshape(B, S, 2, H),
    )


# revision 10
# speedup vs baseline: 1.0423x; 1.0423x over previous
"""MoE routing kernel v2.1 for Trainium2 — grouped GEMM, 8 NeuronCores.

Sharding: 2-way expert-parallel x 4-way data-parallel. Core c handles data
group g = c//2 (2048 tokens) and expert half h = c%2 (5 of the 10 experts).
The gate matrix columns (and the expert weights) are host-permuted per core
so local experts are always gating columns 0..4 — one NEFF serves all cores.

Per core, on device:
  1. PE-transpose x -> xT, gating matmuls, softmax, top-2 (max8), one-hot
     top-1/top-2 masks via value equality (all exact fp32), per-tile ranks
     via triangular matmuls (interleaved with gating).
  2. Cross-tile rank offsets: one totals matmul + Hillis-Steele prefix on
     partition 0 + partition_broadcast.
  3. Tokens scatter [token, 2*token+slot] rows into a DRAM tokmap at their
     capacity-row index via indirect DMA (2 batched calls); reading it back
     row-ordered yields gather/scatter indices. Padding rows were prefilled
     with [0, BIG]: they gather token 0 (finite) and scatter out-of-bounds
     (silently skipped).
  4. Per local expert: batched indirect-DMA row gather, PE-transpose,
     grouped GEMM (fp32r, 1 cycle/col) + bias, batched indirect-DMA row
     scatter into the [2*T, H] output. Pair cores write disjoint rows;
     host sums the pair outputs.

Capacity per local expert (C) is sized at build time from a cheap host-side
gating estimate of the actual input (device still computes all routing).
"""

import os

import numpy as np

import concourse.bass as bass
import concourse.mybir as mybir
import concourse.tile as tile
from concourse import bacc
from concourse.bass import ts
from concourse.bass_utils import run_bass_kernel_spmd

f32 = mybir.dt.float32
f32r = mybir.dt.float32r
i32 = mybir.dt.int32

B, S, E, H, NE = 4, 2048, 1024, 2048, 10
NCORES = 8
NGROUP = 4              # data groups
NLOC = NE // 2          # local experts per core = 5
T2 = (B * S) // NGROUP  # tokens per core = 2048
TT = T2 // 128          # token tiles = 16
EJ = E // 128           # contraction subtiles = 8
CGW = 512
CG = H // CGW           # column groups = 4
BIG = 1.0e7

LAST_RESULTS = None


def _build_program(C):
    RC = C // 128  # row chunks per local expert
    nc = bacc.Bacc(None, target_bir_lowering=False, debug=False)

    xs = nc.dram_tensor("xs", [T2, E], f32, kind="ExternalInput")
    gwT = nc.dram_tensor("gwT", [128, EJ, NE], f32, kind="ExternalInput")
    gbB = nc.dram_tensor("gbB", [128, NE], f32, kind="ExternalInput")
    ew5 = nc.dram_tensor("ew5", [NLOC, E, H], f32, kind="ExternalInput")
    ebB5 = nc.dram_tensor("ebB5", [128, NLOC, H], f32, kind="ExternalInput")
    ident = nc.dram_tensor("ident", [128, 128], f32, kind="ExternalInput")
    tri = nc.dram_tensor("tri", [128, 128], f32, kind="ExternalInput")
    ones2d = nc.dram_tensor("ones2d", [128, 128], f32, kind="ExternalInput")
    toki = nc.dram_tensor("toki", [128, TT], f32, kind="ExternalInput")
    lCm1 = nc.dram_tensor("lCm1", [128, NLOC], f32, kind="ExternalInput")
    ow = nc.dram_tensor("ow", [T2, 2], f32, kind="ExternalOutput")
    oe = nc.dram_tensor("oe", [2 * T2, H], f32, kind="ExternalOutput")

    with tile.TileContext(nc) as tc:
        with (
            tc.tile_pool(name="const", bufs=1) as const,
            tc.tile_pool(name="masks", bufs=1) as mp,
            tc.tile_pool(name="route", bufs=1) as rp,
            tc.tile_pool(name="dram", bufs=1, space="DRAM") as dramp,
            tc.tile_pool(name="wp", bufs=3) as wp,
            tc.tile_pool(name="bp", bufs=2) as bp,
        ):
            ident_sb = const.tile([128, 128], f32)
            nc.sync.dma_start(ident_sb[:], ident[:])
            tri_sb = const.tile([128, 128], f32)
            nc.sync.dma_start(tri_sb[:], tri[:])
            ones_sb = const.tile([128, 128], f32)
            nc.sync.dma_start(ones_sb[:], ones2d[:])
            toki_sb = const.tile([128, TT], f32)
            nc.sync.dma_start(toki_sb[:], toki[:])
            lCm1_sb = const.tile([128, NLOC], f32)
            nc.sync.dma_start(lCm1_sb[:], lCm1[:])
            gw_sb = const.tile([128, EJ, NE], f32)
            nc.sync.dma_start(gw_sb[:], gwT[:])
            gb_sb = const.tile([128, NE], f32)
            nc.sync.dma_start(gb_sb[:], gbB[:])

            tokmap = dramp.tile([NLOC * C, 2], f32)
            # prefill: token=0 (valid), dst=BIG (scatter-skipped)
            fill = rp.tile([128, NLOC * C // 128, 2], f32, tag="fill")
            nc.vector.memset(fill[:, :, 0:1], 0.0)
            nc.vector.memset(fill[:, :, 1:2], BIG)
            nc.sync.dma_start(
                tokmap[:].rearrange("(c p) v -> p c v", p=128), fill[:]
            )
            # vv[k][:, tt, :] = [token, 2*token + k]
            vvk = []
            for k in range(2):
                vv = rp.tile([128, TT, 2], f32, tag=f"vv{k}")
                nc.vector.tensor_copy(vv[:, :, 0:1], toki_sb[:, :, None])
                nc.vector.tensor_scalar(
                    vv[:, :, 1:2], toki_sb[:, :, None], 2.0, float(k),
                    op0=mybir.AluOpType.mult, op1=mybir.AluOpType.add,
                )
                vvk.append(vv)
            offs_cur = rp.tile([1, NE], f32, tag="offs_cur")
            nc.vector.memset(offs_cur[:], 0.0)

            m0f = mp.tile([128, TT, NE], f32)
            m1f = mp.tile([128, TT, NE], f32)
            m01 = mp.tile([128, TT, NE], f32)
            wstage = mp.tile([128, TT, 2], f32)
            tok32 = rp.tile([128, NLOC, RC], i32)  # gather row -> source token
            dst32 = rp.tile([128, NLOC, RC], i32)  # gather row -> output row

            # ===== phase A: gating / top-2 / per-tile ranks =====
            with (
                tc.tile_pool(name="xT", bufs=1) as xtp,
                tc.tile_pool(name="xin", bufs=3) as xpool,
                tc.tile_pool(name="tpsA", bufs=3, space="PSUM") as tpsA,
                tc.tile_pool(name="gps", bufs=2, space="PSUM") as gps_p,
                tc.tile_pool(name="cps", bufs=1, space="PSUM") as cps_p,
                tc.tile_pool(name="offs", bufs=1, space="PSUM") as offs_p,
                tc.tile_pool(name="smx", bufs=2) as spool,
            ):
                xT_sb = xtp.tile([128, EJ, T2], f32)
                offs_ps = offs_p.tile([128, NE], f32)
                # ~5us of dummy matmuls to warm the PE clock gate (HAM)
                wps = tpsA.tile([128, 4, 128], f32, tag="ps")
                for _ in range(12):
                    nc.tensor.matmul(
                        wps[:, 0, :], lhsT=ones_sb[:], rhs=ones_sb[:],
                        start=True, stop=True, skip_group_check=True,
                    )
                for tt in range(TT):
                    xt = xpool.tile([128, E], f32)
                    nc.sync.dma_start(xt[:], xs[ts(tt, 128), :])
                    for jh in range(2):
                        ps = tpsA.tile([128, 4, 128], f32)
                        for jj in range(4):
                            nc.tensor.transpose(
                                ps[:, jj, :], xt[:, ts(jh * 4 + jj, 128)],
                                ident_sb[:],
                            )
                        nc.vector.tensor_copy(
                            xT_sb[:, jh * 4 : jh * 4 + 4, ts(tt, 128)], ps[:]
                        )
                    gps = gps_p.tile([128, NE], f32)
                    # bias via K=1 matmul, then accumulate the gating products
                    nc.tensor.matmul(
                        gps[:], lhsT=ones_sb[0:1, :], rhs=gb_sb[0:1, :],
                        start=True, stop=False,
                    )
                    for j in range(EJ):
                        nc.tensor.matmul(
                            gps[:],
                            lhsT=xT_sb[:, j, ts(tt, 128)],
                            rhs=gw_sb[:, j, :],
                            start=False,
                            stop=(j == EJ - 1),
                        )
                    scores = spool.tile([128, NE], f32)
                    nc.vector.tensor_copy(scores[:], gps[:])
                    v8 = spool.tile([128, 8], f32, tag="v8")
                    nc.vector.max(v8[:], scores[:])
                    nc.vector.tensor_tensor(
                        m0f[:, tt, :], scores[:], v8[:, 0:1].to_broadcast([128, NE]),
                        op=mybir.AluOpType.is_equal,
                    )
                    nc.vector.tensor_tensor(
                        m1f[:, tt, :], scores[:], v8[:, 1:2].to_broadcast([128, NE]),
                        op=mybir.AluOpType.is_equal,
                    )
                    nc.vector.tensor_add(m01[:, tt, :], m0f[:, tt, :], m1f[:, tt, :])
                    # top-2 softmax weights in closed form:
                    # w1 = 1/sum(exp(s - v1)), w2 = exp(v2 - v1) * w1
                    nv1 = spool.tile([128, 1], f32, tag="nv1")
                    nc.vector.tensor_scalar_mul(nv1[:], v8[:, 0:1], -1.0)
                    exps = spool.tile([128, NE], f32, tag="exps")
                    ssum = spool.tile([128, 1], f32, tag="ssum")
                    nc.scalar.activation(
                        exps[:],
                        scores[:],
                        mybir.ActivationFunctionType.Exp,
                        bias=nv1[:],
                        scale=1.0,
                        accum_out=ssum[:],
                    )
                    nc.vector.reciprocal(wstage[:, tt, 0:1], ssum[:])
                    d21 = spool.tile([128, 1], f32, tag="d21")
                    nc.vector.tensor_add(d21[:], v8[:, 1:2], nv1[:])
                    e21 = spool.tile([128, 1], f32, tag="e21")
                    nc.scalar.activation(
                        e21[:], d21[:], mybir.ActivationFunctionType.Exp
                    )
                    nc.vector.tensor_tensor(
                        wstage[:, tt, 1:2], e21[:], wstage[:, tt, 0:1],
                        op=mybir.AluOpType.mult,
                    )
                    # within-tile inclusive rank + cross-tile offset (psum)
                    cps = cps_p.tile([128, NE], f32)
                    nc.tensor.matmul(
                        cps[:], lhsT=tri_sb[:], rhs=m01[:, tt, :],
                        start=True, stop=True,
                    )
                    posi_t = spool.tile([128, NE], f32, tag="posi_t")
                    if tt == 0:
                        nc.vector.tensor_copy(posi_t[:], cps[:])
                    else:
                        nc.vector.tensor_add(posi_t[:], cps[:], offs_sb[:])
                    nc.tensor.matmul(
                        offs_ps[:], lhsT=ones_sb[:], rhs=m01[:, tt, :],
                        start=(tt == 0), stop=(tt == TT - 1),
                        skip_group_check=True,
                    )
                    if tt < TT - 1:
                        offs_sb = spool.tile([128, NE], f32, tag="offs_sb")
                        nc.vector.tensor_copy(offs_sb[:], offs_ps[:])
                    # capacity-row index per slot; scatter [tok, dst] to tokmap
                    t1 = spool.tile([128, NLOC], f32, tag="t1")
                    nc.vector.tensor_add(
                        t1[:], posi_t[:, 0:NLOC], lCm1_sb[:]
                    )
                    for k in range(2):
                        mk5 = (m0f if k == 0 else m1f)[:, tt, 0:NLOC]
                        rk = spool.tile([128, 1], f32, tag=f"rk{k}")
                        t2 = spool.tile([128, NLOC], f32, tag=f"t2{k}")
                        nc.vector.tensor_tensor(
                            t2[:], t1[:], mk5, op=mybir.AluOpType.mult
                        )
                        nc.vector.tensor_reduce(
                            rk[:], t2[:], mybir.AxisListType.X,
                            mybir.AluOpType.add,
                        )
                        selk = spool.tile([128, 1], f32, tag=f"selk{k}")
                        nc.vector.tensor_reduce(
                            selk[:], mk5, mybir.AxisListType.X,
                            mybir.AluOpType.add,
                        )
                        gsel = spool.tile([128, 1], f32, tag=f"gsel{k}")
                        nc.vector.tensor_scalar(
                            gsel[:], selk[:], -BIG, BIG,
                            op0=mybir.AluOpType.mult, op1=mybir.AluOpType.add,
                        )
                        nc.vector.tensor_add(rk[:], rk[:], gsel[:])
                        rk32 = spool.tile([128, 1], i32, tag=f"rk32{k}")
                        nc.vector.tensor_copy(rk32[:], rk[:])
                        nc.gpsimd.indirect_dma_start(
                            out=tokmap[:],
                            out_offset=bass.IndirectOffsetOnAxis(
                                ap=rk32[:], axis=0
                            ),
                            in_=vvk[k][:, tt, :],
                            in_offset=None,
                            bounds_check=NLOC * C - 1,
                            oob_is_err=False,
                        )
                nc.sync.dma_start(ow.rearrange("(i p) k -> p i k", p=128), wstage[:])
                # re-warm the PE clock gate during the routing-index bubble
                wps2 = tpsA.tile([128, 4, 128], f32, tag="ps")
                for _ in range(40):
                    nc.tensor.matmul(
                        wps2[:, 0, :], lhsT=ones_sb[:], rhs=ones_sb[:],
                        start=True, stop=True, skip_group_check=True,
                    )

            # ===== phase B: read back tokmap -> (token, dst) int32 =====
            with tc.tile_pool(name="rt", bufs=1) as rtp:
                idxrow = rtp.tile([128, NLOC * RC, 2], f32, tag="idxrow")
                nc.sync.dma_start(
                    idxrow[:], tokmap[:].rearrange("(c p) v -> p c v", p=128)
                )
                nc.vector.tensor_copy(
                    tok32[:].rearrange("p a b -> p (a b)")[:, :, None],
                    idxrow[:, :, 0:1],
                )
                nc.vector.tensor_copy(
                    dst32[:].rearrange("p a b -> p (a b)")[:, :, None],
                    idxrow[:, :, 1:2],
                )

            # ===== phase E: gather -> grouped GEMM -> scatter =====
            with (
                tc.tile_pool(name="xg", bufs=2) as xgp,
                tc.tile_pool(name="xgT", bufs=2) as xgtp,
                tc.tile_pool(name="og", bufs=2) as ogp,
                tc.tile_pool(name="tpsE", bufs=2, space="PSUM") as tpsE,
                tc.tile_pool(name="mps", bufs=6, space="PSUM") as mps,
            ):
                def gather_l(l):
                    xg = xgp.tile([128, RC, E], f32, tag="xg")
                    for rc in range(RC):
                        nc.gpsimd.indirect_dma_start(
                            out=xg[:, rc, :],
                            out_offset=None,
                            in_=xs[:],
                            in_offset=bass.IndirectOffsetOnAxis(
                                ap=tok32[:, l, ts(rc, 1)], axis=0
                            ),
                        )
                    return xg

                def transpose_chunk(xg, xgT, rc):
                    for jh in range(2):
                        ps = tpsE.tile([128, 4, 128], f32)
                        for jj in range(4):
                            nc.tensor.transpose(
                                ps[:, jj, :], xg[:, rc, ts(jh * 4 + jj, 128)],
                                ident_sb[:],
                            )
                        nc.vector.tensor_copy(
                            xgT[:, jh * 4 : jh * 4 + 4, ts(rc, 128)], ps[:]
                        )

                xg_cur = gather_l(0)
                xgT_cur = xgtp.tile([128, EJ, C], f32r, tag="xgT")
                for rc in range(RC):
                    transpose_chunk(xg_cur, xgT_cur, rc)
                for l in range(NLOC):
                    xgT = xgT_cur
                    if l + 1 < NLOC:
                        xg_next = gather_l(l + 1)
                        xgT_next = xgtp.tile([128, EJ, C], f32r, tag="xgT")
                    og = ogp.tile([128, RC, H], f32)
                    for cg in range(CG):
                        wT = wp.tile([128, EJ, CGW], f32r)
                        nc.sync.dma_start(
                            wT[:],
                            ew5[l]
                            .rearrange("(j p) h -> p j h", p=128)[:, :, ts(cg, CGW)]
                            .bitcast(f32r),
                        )
                        bb = bp.tile([128, CGW], f32)
                        nc.sync.dma_start(bb[:], ebB5[:, l, ts(cg, CGW)])
                        for rc in range(RC):
                            ps = mps.tile([128, CGW], f32)
                            for j in range(EJ):
                                nc.tensor.matmul(
                                    ps[:],
                                    lhsT=xgT[:, j, ts(rc, 128)],
                                    rhs=wT[:, j, :],
                                    start=(j == 0),
                                    stop=(j == EJ - 1),
                                )
                            nc.vector.tensor_add(ps[:], ps[:], bb[:])
                            nc.scalar.copy(og[:, rc, ts(cg, CGW)], ps[:])
                        # next expert's transposes ride between matmul bursts
                        if l + 1 < NLOC and cg < RC:
                            transpose_chunk(xg_next, xgT_next, cg)
                    for rc in range(RC):
                        nc.gpsimd.indirect_dma_start(
                            out=oe[:],
                            out_offset=bass.IndirectOffsetOnAxis(
                                ap=dst32[:, l, ts(rc, 1)], axis=0
                            ),
                            in_=og[:, rc, :],
                            in_offset=None,
                            bounds_check=2 * T2 - 1,
                            oob_is_err=False,
                        )
                    if l + 1 < NLOC:
                        xg_cur = xg_next
                        xgT_cur = xgT_next

    nc.compile()
    return nc


_CACHED = {}


def _capacity(x, gate_w, gate_b):
    """Host-side gating estimate, only to size the static capacity."""
    xf = x.reshape(B * S, E)
    scores = xf @ gate_w + gate_b
    part = np.argpartition(-scores, 2, axis=1)[:, :2]
    maxc = 0
    for g in range(NGROUP):
        sl = part[g * T2 : (g + 1) * T2]
        cnt = np.bincount(sl.ravel(), minlength=NE)
        maxc = max(maxc, int(cnt.max()))
    return ((maxc + 45 + 127) // 128) * 128


def kernel(x, gate_w, gate_b, expert_w, expert_b):
    global LAST_RESULTS
    x = np.ascontiguousarray(np.asarray(x, dtype=np.float32))
    gate_w = np.ascontiguousarray(np.asarray(gate_w, dtype=np.float32))
    gate_b = np.ascontiguousarray(np.asarray(gate_b, dtype=np.float32))
    expert_w = np.ascontiguousarray(np.asarray(expert_w, dtype=np.float32))
    expert_b = np.ascontiguousarray(np.asarray(expert_b, dtype=np.float32))

    C = _capacity(x, gate_w, gate_b)
    if C not in _CACHED:
        _CACHED[C] = _build_program(C)
    nc = _CACHED[C]

    xf = x.reshape(B * S, E)
    identity = np.eye(128, dtype=np.float32)
    tri = np.triu(np.ones((128, 128), dtype=np.float32))  # tri[t', t]=1 if t'<=t
    ones2d = np.ones((128, 128), dtype=np.float32)
    lCm1 = np.broadcast_to(
        (np.arange(NLOC, dtype=np.float32) * C - 1.0)[None, :], (128, NLOC)
    ).copy()
    toki = np.empty((128, TT), dtype=np.float32)
    for tt in range(TT):
        toki[:, tt] = tt * 128 + np.arange(128)

    in_maps = []
    for c in range(NCORES):
        g, h = divmod(c, 2)
        perm = list(range(NE)) if h == 0 else list(range(NLOC, NE)) + list(
            range(NLOC)
        )
        gw_p = gate_w[:, perm]
        gwT = np.ascontiguousarray(gw_p.reshape(EJ, 128, NE).transpose(1, 0, 2))
        gbB = np.ascontiguousarray(
            np.broadcast_to(gate_b[perm][None, :], (128, NE))
        )
        ew5 = np.ascontiguousarray(expert_w[perm[:NLOC]])
        ebB5 = np.ascontiguousarray(
            np.broadcast_to(expert_b[perm[:NLOC]][None, :, :], (128, NLOC, H))
        )
        in_maps.append(
            {
                "xs": np.ascontiguousarray(xf[g * T2 : (g + 1) * T2]),
                "gwT": gwT,
                "gbB": gbB,
                "ew5": ew5,
                "ebB5": ebB5,
                "ident": identity,
                "tri": tri,
                "ones2d": ones2d,
                "toki": toki,
                "lCm1": lCm1,
            }
        )

    trace = bool(int(os.environ.get("KERNEL_TRACE", "0")))
    res = run_bass_kernel_spmd(
        nc,
        in_maps,
        core_ids=list(range(NCORES)),
        trace=trace,
        trace_cores=list(range(NCORES)) if trace else None,
    )
    LAST_RESULTS = res

    ow_full = np.concatenate(
        [res.results[2 * g]["ow"] for g in range(NGROUP)], axis=0
    )
    oe_full = np.concatenate(
        [
            (res.results[2 * g]["oe"] + res.results[2 * g + 1]["oe"]).reshape(
                T2, 2, H
            )
            for g in range(NGROUP)
        ],
        axis=0,
    )
    return (
        ow_full.reshape(B, S, 2),
        oe_full.reshape(B, S, 2, H),
    )
